# revision 38
# baseline (speedup 1.0000x reference)
"""ConvSquare Trainium2 kernel (fp8 DoubleRow hi/lo formulation).

Math: out = conv2d_3x3(x * p, weight) + bias, stride 1, pad 1, where
p = (a*alpha + b)*alpha + c on the zero-padded alpha field.

Sharding: 8 cores = batch(4) x row-half(2); each core emits [64, 64, 128].

Device pipeline per core:
  - Host precomputes y = x*p (elementwise prep, 0.01% of FLOPs) and splits
    it into fp8e4m3 hi + lo residual streams, packed with a row pitch of
    136 so the DoubleRow Ko stride (2 rows = 272 elems) is 16-aligned.
    Partitions 0-63 hold rows 0..67, partitions 64-127 hold rows 1..68.
  - Weights are scaled x16 and split hi/lo in fp8; the ACT PSUM->SBUF
    copy unscales via its activation `scale` and adds bias.
  - One DoubleRow matmul contracts 4 taps at once: partition halves give
    row shifts {0,1}, the Ko pair dim (+272 elems = +2 rows) gives {2,3}
    (tap k=3 has zero weight). 3 DR matmuls cover the 9 taps per pass;
    3 passes (wh*yh, wh*yl, wl*yh) give ~bf16 accuracy at 0.5 cyc/row:
    9 DR matmuls x 128-free per output row = 576 PE cycles vs 768 bf16.
  - Small first/last row groups, grouped stores fanned across queues,
    two warm-up matmuls to open the PE clock-ramp window early.
"""

import sys

import numpy as np

sys.path.insert(0, "/opt/trn_rl_repo")

import ml_dtypes

import concourse.bass as bass
import concourse.mybir as mybir
from concourse.bass_utils import run_bass_kernel_spmd
from concourse.tile import TileContext

F32 = mybir.dt.float32
BF16 = mybir.dt.bfloat16
FP8 = mybir.dt.float8e4

B, C, O, H, W = 4, 64, 64, 128, 128
HS = 64  # output rows per core
WP2 = 136  # padded row pitch (130 data cols, padded so 2 rows % 16 == 0)
NR = 68  # rows per stored half-slab
FREE3 = NR * WP2  # 9248
KO = 2 * WP2  # 272: DoubleRow Ko stride (+2 rows)
WSCALE = 16.0
N_WARM = 2
# y-stream chunk edges in rows (both hi and lo streams)
ROW_EDGES = [0, 6, 14, 22, 30, 38, 46, 56, 68]
# matmul groups: (start_row, n_rows)
MM_CHUNKS = (
    [(0, 2), (2, 2), (4, 4)]
    + [(8 + 4 * i, 4) for i in range(13)]
    + [(62, 2), (60, 2)]
)
STORE_GROUPS = [
    (0, 3072),
    (3072, 5120),
    (5120, 6656),
    (6656, 7680),
    (7680, 7936),
]

_cache: dict = {}


def _program() -> bass.Bass:
    from concourse.bacc import Bacc

    nc = Bacc()
    # yhl packs the fp8 hi stream (cols 0:FREE3) and lo stream (FREE3:)
    yhl_h = nc.dram_tensor("yhl", [128, 2 * FREE3], FP8, kind="ExternalInput")
    # w8: blocks (s, l): cols (s*3+l)*128 + two*64 + o; s=0 hi, s=1 lo
    w_h = nc.dram_tensor("w", [128, 768], FP8, kind="ExternalInput")
    bias_h = nc.dram_tensor("bias", [O, 1], F32, kind="ExternalInput")
    out_h = nc.dram_tensor("out", [O, HS * W], F32, kind="ExternalOutput")

    with TileContext(nc) as tc:
        with (
            tc.tile_pool(name="const", bufs=1) as cpool,
            tc.tile_pool(name="work", bufs=1) as wpool,
            tc.tile_pool(name="psum", bufs=4, space="PSUM") as ppool,
        ):
            # PE warm-up to start the clock-ramp window early
            wrm_w = cpool.tile([1, 1], BF16)
            wrm_r = cpool.tile([1, 512], BF16)
            nc.gpsimd.memset(wrm_w[:, :], 0.0)
            nc.vector.memset(wrm_r[:, :], 0.0)
            for _ in range(N_WARM):
                pw = ppool.tile([O, 512], F32)
                nc.tensor.matmul(
                    pw[0:1, :], wrm_w[:, :], wrm_r[:, :], start=True, stop=True
                )

            wt = cpool.tile([128, 768], FP8)
            bt = cpool.tile([O, 1], F32)
            yhl = wpool.tile([128, 2 * FREE3], FP8)
            st = wpool.tile([O, HS * W], F32)

            # weights/bias via Pool SWDGE (no HWDGE slot)
            nc.gpsimd.dma_start(out=wt[:, :], in_=w_h[:, :])
            nc.gpsimd.dma_start(out=bt[:, :], in_=bias_h[:, :])

            yhl3 = yhl[:].rearrange("p (s c) -> p s c", s=2)
            for j in range(len(ROW_EDGES) - 1):
                c0 = ROW_EDGES[j] * WP2
                c1 = ROW_EDGES[j + 1] * WP2
                # one DMA per chunk loads the hi and lo slices together
                nc.sync.dma_start(
                    out=yhl3[0:128, 0:2, c0:c1],
                    in_=bass.AP(
                        tensor=yhl_h[:, :].tensor,
                        offset=c0,
                        ap=[[2 * FREE3, 128], [FREE3, 2], [1, c1 - c0]],
                    ),
                )

            # lhsT blocks: [p, block (s*3+l), two, o]
            w4 = wt[:].rearrange("p (b two o) -> p b two o", b=6, two=2)
            yh3 = yhl[:, 0:FREE3].rearrange("p (r c) -> p r c", r=NR)
            yl3 = yhl[:, FREE3 : 2 * FREE3].rearrange("p (r c) -> p r c", r=NR)

            def rhs(stream3, m, l):
                # rows {m, m+2} via step-2 slice = the DoubleRow Ko pair
                return stream3[0:128, m : m + 3 : 2, l : l + W]

            passes = ((0, yh3), (0, yl3), (1, yh3))
            for R, r in MM_CHUNKS:
                ps = ppool.tile([O, r * W], F32)
                for i in range(r):
                    m = R + i
                    sl = ps[:, W * i : W * (i + 1)]
                    n9 = 0
                    for s, ystream in passes:
                        for l in range(3):
                            nc.tensor.matmul(
                                sl,
                                w4[0:128, 3 * s + l, 0:2, 0:64],
                                rhs(ystream, m, l),
                                start=(n9 == 0),
                                stop=(n9 == 8),
                                perf_mode=mybir.MatmulPerfMode.DoubleRow,
                            )
                            n9 += 1
                # unscale (1/16) + bias while copying PSUM -> SBUF staging
                ss = st[:, W * R : W * (R + r)]
                if R == 62:
                    nc.vector.tensor_scalar(
                        out=ss, in0=ps[:, :], scalar1=1.0 / WSCALE,
                        scalar2=bt[:, 0:1], op0=mybir.AluOpType.mult,
                        op1=mybir.AluOpType.add,
                    )
                else:
                    nc.scalar.activation(
                        ss, ps[:, :], mybir.ActivationFunctionType.Identity,
                        bias=bt[:, 0:1], scale=1.0 / WSCALE,
                    )
            for g0, g1 in STORE_GROUPS:
                nc.sync.dma_start(out=out_h[:, g0:g1], in_=st[:, g0:g1])
            # tail stores on different queues so their issue overlaps
            nc.gpsimd.dma_start(out=out_h[:, 7936:8064], in_=st[:, 7936:8064])
            nc.scalar.dma_start(out=out_h[:, 8064:8192], in_=st[:, 8064:8192])
    return nc


def _pack_weights(wt):
    """[O,C,3,3] -> [128, 768] fp8: blocks (s,l), s=0: fp8(16w) hi,
    s=1: fp8 residual; partition p = channel p%64 with row-shift p//64;
    two-slot j selects tap k = p//64 + 2j (k=3 -> 0)."""
    w16 = wt.astype(np.float64).transpose(1, 2, 3, 0) * WSCALE  # [c,k,l,o]
    wk = np.zeros((C, 4, 3, O), np.float64)
    wk[:, 0:3] = w16
    hi = np.asarray(wk.astype(ml_dtypes.float8_e4m3), np.float64)
    lo = (wk - hi).astype(ml_dtypes.float8_e4m3)
    out = np.zeros((128, 768), ml_dtypes.float8_e4m3)
    for s, ww in ((0, hi.astype(ml_dtypes.float8_e4m3)), (1, lo)):
        for l in range(3):
            for h in range(2):
                for j in range(2):
                    out[64 * h : 64 * h + 64,
                        (3 * s + l) * 128 + 64 * j : (3 * s + l) * 128 + 64 * j + 64,
                        ] = np.asarray(ww)[:, h + 2 * j, l, :]
    return np.ascontiguousarray(out)


def kernel(inputs, alpha, weight, bias, a, b, c):
    x = np.asarray(inputs, np.float32)
    al = np.asarray(alpha, np.float32)
    wt = np.asarray(weight, np.float32)
    bs = np.asarray(bias, np.float32)
    av, bv, cv = float(a), float(b), float(c)

    if "nc" not in _cache:
        nc_new = _program()
        nc_new.finalize()
        _cache["nc"] = nc_new
    nc = _cache["nc"]

    w_packed = _pack_weights(wt)
    b_packed = np.ascontiguousarray(bs.reshape(O, 1))

    in_maps = []
    for core in range(8):
        b_idx, hh = divmod(core, 2)
        r0 = hh * HS - 1  # global row of padded row 0
        ys = np.zeros((C, NR + 1, WP2), np.float32)
        als = np.zeros((1, NR + 1, WP2), np.float32)
        lo_r = max(0, r0)
        hi_r = min(H, r0 + HS + 2)
        ys[:, lo_r - r0 : hi_r - r0, 1 : 1 + W] = x[b_idx, :, lo_r:hi_r, :]
        als[:, lo_r - r0 : hi_r - r0, 1 : 1 + W] = al[b_idx, :, lo_r:hi_r, :]
        y = ys * ((av * als + bv) * als + cv)  # y = x * p
        y_hi = y.astype(ml_dtypes.float8_e4m3)
        y_lo = (y - np.asarray(y_hi, np.float32)).astype(ml_dtypes.float8_e4m3)

        def pack(yv):
            return np.concatenate(
                [yv[:, 0:NR].reshape(C, FREE3), yv[:, 1 : NR + 1].reshape(C, FREE3)],
                axis=0,
            )

        in_maps.append(
            {
                "yhl": np.ascontiguousarray(
                    np.concatenate([pack(y_hi), pack(y_lo)], axis=1)
                ),
                "w": w_packed,
                "bias": b_packed,
            }
        )

    res = run_bass_kernel_spmd(nc, in_maps, list(range(8)))

    out = np.empty((B, O, H, W), np.float32)
    for core in range(8):
        b_idx, hh = divmod(core, 2)
        out[b_idx, :, hh * HS : (hh + 1) * HS, :] = res.results[core]["out"].reshape(
            O, HS, W
        )
    return out


# revision 39
# speedup vs baseline: 1.0086x; 1.0086x over previous
"""ConvSquare Trainium2 kernel (fp8 DoubleRow hi/lo formulation).

Math: out = conv2d_3x3(x * p, weight) + bias, stride 1, pad 1, where
p = (a*alpha + b)*alpha + c on the zero-padded alpha field.

Sharding: 8 cores = batch(4) x row-half(2); each core emits [64, 64, 128].

Device pipeline per core:
  - Host precomputes y = x*p (elementwise prep, 0.01% of FLOPs) and splits
    it into fp8e4m3 hi + lo residual streams, packed with a row pitch of
    136 so the DoubleRow Ko stride (2 rows = 272 elems) is 16-aligned.
    Partitions 0-63 hold rows 0..67, partitions 64-127 hold rows 1..68.
  - Weights are scaled x16 and split hi/lo in fp8; the ACT PSUM->SBUF
    copy unscales via its activation `scale` and adds bias.
  - One DoubleRow matmul contracts 4 taps at once: partition halves give
    row shifts {0,1}, the Ko pair dim (+272 elems = +2 rows) gives {2,3}
    (tap k=3 has zero weight). 3 DR matmuls cover the 9 taps per pass;
    3 passes (wh*yh, wh*yl, wl*yh) give ~bf16 accuracy at 0.5 cyc/row:
    9 DR matmuls x 128-free per output row = 576 PE cycles vs 768 bf16.
  - Small first/last row groups, grouped stores fanned across queues,
    two warm-up matmuls to open the PE clock-ramp window early.
"""

import sys

import numpy as np

sys.path.insert(0, "/opt/trn_rl_repo")

import ml_dtypes

import concourse.bass as bass
import concourse.mybir as mybir
from concourse.bass_utils import run_bass_kernel_spmd
from concourse.tile import TileContext

F32 = mybir.dt.float32
BF16 = mybir.dt.bfloat16
FP8 = mybir.dt.float8e4

B, C, O, H, W = 4, 64, 64, 128, 128
HS = 64  # output rows per core
WP2 = 136  # padded row pitch (130 data cols, padded so 2 rows % 16 == 0)
NR = 68  # rows per stored half-slab
FREE3 = NR * WP2  # 9248
KO = 2 * WP2  # 272: DoubleRow Ko stride (+2 rows)
WSCALE = 16.0
N_WARM = 2
# y-stream chunk edges in rows (both hi and lo streams)
ROW_EDGES = [0, 6, 14, 22, 30, 38, 46, 56, 68]
# matmul groups: (start_row, n_rows)
MM_CHUNKS = (
    [(0, 2), (2, 2), (4, 4)]
    + [(8 + 4 * i, 4) for i in range(13)]
    + [(62, 2), (60, 2)]
)
STORE_GROUPS = [
    (0, 3072),
    (3072, 5120),
    (5120, 6656),
    (6656, 7680),
    (7680, 7936),
]

_cache: dict = {}


def _program() -> bass.Bass:
    from concourse.bacc import Bacc

    nc = Bacc()
    yh_h = nc.dram_tensor("yh", [128, FREE3], FP8, kind="ExternalInput")
    yl_h = nc.dram_tensor("yl", [128, FREE3], FP8, kind="ExternalInput")
    # w8: blocks (s, l): cols (s*3+l)*128 + two*64 + o; s=0 hi, s=1 lo
    w_h = nc.dram_tensor("w", [128, 768], FP8, kind="ExternalInput")
    bias_h = nc.dram_tensor("bias", [O, 1], F32, kind="ExternalInput")
    out_h = nc.dram_tensor("out", [O, HS * W], F32, kind="ExternalOutput")

    with TileContext(nc) as tc:
        with (
            tc.tile_pool(name="const", bufs=1) as cpool,
            tc.tile_pool(name="work", bufs=1) as wpool,
            tc.tile_pool(name="psum", bufs=4, space="PSUM") as ppool,
        ):
            # PE warm-up to start the clock-ramp window early
            wrm_w = cpool.tile([1, 1], BF16)
            wrm_r = cpool.tile([1, 512], BF16)
            nc.gpsimd.memset(wrm_w[:, :], 0.0)
            nc.vector.memset(wrm_r[:, :], 0.0)
            for _ in range(N_WARM):
                pw = ppool.tile([O, 512], F32)
                nc.tensor.matmul(
                    pw[0:1, :], wrm_w[:, :], wrm_r[:, :], start=True, stop=True
                )

            wt = cpool.tile([128, 768], FP8)
            bt = cpool.tile([O, 1], F32)
            yh = wpool.tile([128, FREE3], FP8)
            yl = wpool.tile([128, FREE3], FP8)
            st = wpool.tile([O, HS * W], F32)

            # weights/bias via Pool SWDGE (no HWDGE slot)
            nc.gpsimd.dma_start(out=wt[:, :], in_=w_h[:, :])
            nc.gpsimd.dma_start(out=bt[:, :], in_=bias_h[:, :])

            for j in range(len(ROW_EDGES) - 1):
                c0 = ROW_EDGES[j] * WP2
                c1 = ROW_EDGES[j + 1] * WP2
                nc.sync.dma_start(out=yh[:, c0:c1], in_=yh_h[:, c0:c1])
                nc.sync.dma_start(out=yl[:, c0:c1], in_=yl_h[:, c0:c1])

            # lhsT blocks: [p, block (s*3+l), two, o]
            w4 = wt[:].rearrange("p (b two o) -> p b two o", b=6, two=2)
            yh3 = yh[:].rearrange("p (r c) -> p r c", r=NR)
            yl3 = yl[:].rearrange("p (r c) -> p r c", r=NR)

            def rhs(stream3, m, l):
                # rows {m, m+2} via step-2 slice = the DoubleRow Ko pair
                return stream3[0:128, m : m + 3 : 2, l : l + W]

            passes = ((0, yh3), (0, yl3), (1, yh3))
            for R, r in MM_CHUNKS:
                ps = ppool.tile([O, r * W], F32)
                for i in range(r):
                    m = R + i
                    sl = ps[:, W * i : W * (i + 1)]
                    n9 = 0
                    for s, ystream in passes:
                        for l in range(3):
                            nc.tensor.matmul(
                                sl,
                                w4[0:128, 3 * s + l, 0:2, 0:64],
                                rhs(ystream, m, l),
                                start=(n9 == 0),
                                stop=(n9 == 8),
                                perf_mode=mybir.MatmulPerfMode.DoubleRow,
                            )
                            n9 += 1
                # unscale (1/16) + bias while copying PSUM -> SBUF staging
                ss = st[:, W * R : W * (R + r)]
                if R == 62:
                    nc.vector.tensor_scalar(
                        out=ss, in0=ps[:, :], scalar1=1.0 / WSCALE,
                        scalar2=bt[:, 0:1], op0=mybir.AluOpType.mult,
                        op1=mybir.AluOpType.add,
                    )
                else:
                    nc.scalar.activation(
                        ss, ps[:, :], mybir.ActivationFunctionType.Identity,
                        bias=bt[:, 0:1], scale=1.0 / WSCALE,
                    )
            for g0, g1 in STORE_GROUPS:
                nc.sync.dma_start(out=out_h[:, g0:g1], in_=st[:, g0:g1])
            # tail stores on different queues so their issue overlaps
            nc.gpsimd.dma_start(out=out_h[:, 7936:8064], in_=st[:, 7936:8064])
            nc.scalar.dma_start(out=out_h[:, 8064:8192], in_=st[:, 8064:8192])
    return nc


def _pack_weights(wt):
    """[O,C,3,3] -> [128, 768] fp8: blocks (s,l), s=0: fp8(16w) hi,
    s=1: fp8 residual; partition p = channel p%64 with row-shift p//64;
    two-slot j selects tap k = p//64 + 2j (k=3 -> 0)."""
    w16 = wt.astype(np.float64).transpose(1, 2, 3, 0) * WSCALE  # [c,k,l,o]
    wk = np.zeros((C, 4, 3, O), np.float64)
    wk[:, 0:3] = w16
    hi = np.asarray(wk.astype(ml_dtypes.float8_e4m3), np.float64)
    lo = (wk - hi).astype(ml_dtypes.float8_e4m3)
    out = np.zeros((128, 768), ml_dtypes.float8_e4m3)
    for s, ww in ((0, hi.astype(ml_dtypes.float8_e4m3)), (1, lo)):
        for l in range(3):
            for h in range(2):
                for j in range(2):
                    out[64 * h : 64 * h + 64,
                        (3 * s + l) * 128 + 64 * j : (3 * s + l) * 128 + 64 * j + 64,
                        ] = np.asarray(ww)[:, h + 2 * j, l, :]
    return np.ascontiguousarray(out)


def kernel(inputs, alpha, weight, bias, a, b, c):
    x = np.asarray(inputs, np.float32)
    al = np.asarray(alpha, np.float32)
    wt = np.asarray(weight, np.float32)
    bs = np.asarray(bias, np.float32)
    av, bv, cv = float(a), float(b), float(c)

    if "nc" not in _cache:
        nc_new = _program()
        nc_new.finalize()
        _cache["nc"] = nc_new
    nc = _cache["nc"]

    w_packed = _pack_weights(wt)
    b_packed = np.ascontiguousarray(bs.reshape(O, 1))

    in_maps = []
    for core in range(8):
        b_idx, hh = divmod(core, 2)
        r0 = hh * HS - 1  # global row of padded row 0
        ys = np.zeros((C, NR + 1, WP2), np.float32)
        als = np.zeros((1, NR + 1, WP2), np.float32)
        lo_r = max(0, r0)
        hi_r = min(H, r0 + HS + 2)
        ys[:, lo_r - r0 : hi_r - r0, 1 : 1 + W] = x[b_idx, :, lo_r:hi_r, :]
        als[:, lo_r - r0 : hi_r - r0, 1 : 1 + W] = al[b_idx, :, lo_r:hi_r, :]
        y = ys * ((av * als + bv) * als + cv)  # y = x * p
        y_hi = y.astype(ml_dtypes.float8_e4m3)
        y_lo = (y - np.asarray(y_hi, np.float32)).astype(ml_dtypes.float8_e4m3)

        def pack(yv):
            return np.concatenate(
                [yv[:, 0:NR].reshape(C, FREE3), yv[:, 1 : NR + 1].reshape(C, FREE3)],
                axis=0,
            )

        in_maps.append(
            {
                "yh": np.ascontiguousarray(pack(y_hi)),
                "yl": np.ascontiguousarray(pack(y_lo)),
                "w": w_packed,
                "bias": b_packed,
            }
        )

    res = run_bass_kernel_spmd(nc, in_maps, list(range(8)))

    out = np.empty((B, O, H, W), np.float32)
    for core in range(8):
        b_idx, hh = divmod(core, 2)
        out[b_idx, :, hh * HS : (hh + 1) * HS, :] = res.results[core]["out"].reshape(
            O, HS, W
        )
    return out


# revision 40
# speedup vs baseline: 1.0127x; 1.0041x over previous
"""ConvSquare Trainium2 kernel (fp8 DoubleRow hi/lo formulation).

Math: out = conv2d_3x3(x * p, weight) + bias, stride 1, pad 1, where
p = (a*alpha + b)*alpha + c on the zero-padded alpha field.

Sharding: 8 cores = batch(4) x row-half(2); each core emits [64, 64, 128].

Device pipeline per core:
  - Host precomputes y = x*p (elementwise prep, 0.01% of FLOPs) and splits
    it into fp8e4m3 hi + lo residual streams, packed with a row pitch of
    136 so the DoubleRow Ko stride (2 rows = 272 elems) is 16-aligned.
    Partitions 0-63 hold rows 0..67, partitions 64-127 hold rows 1..68.
  - Weights are scaled x16 and split hi/lo in fp8; the ACT PSUM->SBUF
    copy unscales via its activation `scale` and adds bias.
  - One DoubleRow matmul contracts 4 taps at once: partition halves give
    row shifts {0,1}, the Ko pair dim (+272 elems = +2 rows) gives {2,3}
    (tap k=3 has zero weight). 3 DR matmuls cover the 9 taps per pass;
    3 passes (wh*yh, wh*yl, wl*yh) give ~bf16 accuracy at 0.5 cyc/row:
    9 DR matmuls x 128-free per output row = 576 PE cycles vs 768 bf16.
  - Small first/last row groups, grouped stores fanned across queues,
    two warm-up matmuls to open the PE clock-ramp window early.
"""

import sys

import numpy as np

sys.path.insert(0, "/opt/trn_rl_repo")

import ml_dtypes

import concourse.bass as bass
import concourse.mybir as mybir
from concourse.bass_utils import run_bass_kernel_spmd
from concourse.tile import TileContext

F32 = mybir.dt.float32
BF16 = mybir.dt.bfloat16
FP8 = mybir.dt.float8e4

B, C, O, H, W = 4, 64, 64, 128, 128
HS = 64  # output rows per core
WP2 = 136  # padded row pitch (130 data cols, padded so 2 rows % 16 == 0)
NR = 68  # rows per stored half-slab
FREE3 = NR * WP2  # 9248
KO = 2 * WP2  # 272: DoubleRow Ko stride (+2 rows)
WSCALE = 16.0
N_WARM = 2
# y-stream chunk edges in rows (both hi and lo streams)
ROW_EDGES = [0, 6, 12, 20, 28, 38, 48, 58, 68]
# matmul groups: (start_row, n_rows)
MM_CHUNKS = (
    [(0, 2), (2, 2), (4, 4)]
    + [(8 + 4 * i, 4) for i in range(13)]
    + [(62, 2), (60, 2)]
)
STORE_GROUPS = [
    (0, 3072),
    (3072, 5120),
    (5120, 6656),
    (6656, 7680),
    (7680, 7936),
]

_cache: dict = {}


def _program() -> bass.Bass:
    from concourse.bacc import Bacc

    nc = Bacc()
    yh_h = nc.dram_tensor("yh", [128, FREE3], FP8, kind="ExternalInput")
    yl_h = nc.dram_tensor("yl", [128, FREE3], FP8, kind="ExternalInput")
    # w8: blocks (s, l): cols (s*3+l)*128 + two*64 + o; s=0 hi, s=1 lo
    w_h = nc.dram_tensor("w", [128, 768], FP8, kind="ExternalInput")
    bias_h = nc.dram_tensor("bias", [O, 1], F32, kind="ExternalInput")
    out_h = nc.dram_tensor("out", [O, HS * W], F32, kind="ExternalOutput")

    with TileContext(nc) as tc:
        with (
            tc.tile_pool(name="const", bufs=1) as cpool,
            tc.tile_pool(name="work", bufs=1) as wpool,
            tc.tile_pool(name="psum", bufs=4, space="PSUM") as ppool,
        ):
            # PE warm-up to start the clock-ramp window early
            wrm_w = cpool.tile([1, 1], BF16)
            wrm_r = cpool.tile([1, 512], BF16)
            nc.gpsimd.memset(wrm_w[:, :], 0.0)
            nc.vector.memset(wrm_r[:, :], 0.0)
            for _ in range(N_WARM):
                pw = ppool.tile([O, 512], F32)
                nc.tensor.matmul(
                    pw[0:1, :], wrm_w[:, :], wrm_r[:, :], start=True, stop=True
                )

            wt = cpool.tile([128, 768], FP8)
            bt = cpool.tile([O, 1], F32)
            yh = wpool.tile([128, FREE3], FP8)
            yl = wpool.tile([128, FREE3], FP8)
            st = wpool.tile([O, HS * W], F32)

            # weights/bias via Pool SWDGE (no HWDGE slot)
            nc.gpsimd.dma_start(out=wt[:, :], in_=w_h[:, :])
            nc.gpsimd.dma_start(out=bt[:, :], in_=bias_h[:, :])

            for j in range(len(ROW_EDGES) - 1):
                c0 = ROW_EDGES[j] * WP2
                c1 = ROW_EDGES[j + 1] * WP2
                nc.sync.dma_start(out=yh[:, c0:c1], in_=yh_h[:, c0:c1])
                nc.sync.dma_start(out=yl[:, c0:c1], in_=yl_h[:, c0:c1])

            # lhsT blocks: [p, block (s*3+l), two, o]
            w4 = wt[:].rearrange("p (b two o) -> p b two o", b=6, two=2)
            yh3 = yh[:].rearrange("p (r c) -> p r c", r=NR)
            yl3 = yl[:].rearrange("p (r c) -> p r c", r=NR)

            def rhs(stream3, m, l):
                # rows {m, m+2} via step-2 slice = the DoubleRow Ko pair
                return stream3[0:128, m : m + 3 : 2, l : l + W]

            passes = ((0, yh3), (0, yl3), (1, yh3))
            for R, r in MM_CHUNKS:
                ps = ppool.tile([O, r * W], F32)
                for i in range(r):
                    m = R + i
                    sl = ps[:, W * i : W * (i + 1)]
                    n9 = 0
                    for s, ystream in passes:
                        for l in range(3):
                            nc.tensor.matmul(
                                sl,
                                w4[0:128, 3 * s + l, 0:2, 0:64],
                                rhs(ystream, m, l),
                                start=(n9 == 0),
                                stop=(n9 == 8),
                                perf_mode=mybir.MatmulPerfMode.DoubleRow,
                            )
                            n9 += 1
                # unscale (1/16) + bias while copying PSUM -> SBUF staging
                ss = st[:, W * R : W * (R + r)]
                if R == 62:
                    nc.vector.tensor_scalar(
                        out=ss, in0=ps[:, :], scalar1=1.0 / WSCALE,
                        scalar2=bt[:, 0:1], op0=mybir.AluOpType.mult,
                        op1=mybir.AluOpType.add,
                    )
                else:
                    nc.scalar.activation(
                        ss, ps[:, :], mybir.ActivationFunctionType.Identity,
                        bias=bt[:, 0:1], scale=1.0 / WSCALE,
                    )
            for g0, g1 in STORE_GROUPS:
                nc.sync.dma_start(out=out_h[:, g0:g1], in_=st[:, g0:g1])
            # tail stores on different queues so their issue overlaps
            nc.gpsimd.dma_start(out=out_h[:, 7936:8064], in_=st[:, 7936:8064])
            nc.scalar.dma_start(out=out_h[:, 8064:8192], in_=st[:, 8064:8192])
    return nc


def _pack_weights(wt):
    """[O,C,3,3] -> [128, 768] fp8: blocks (s,l), s=0: fp8(16w) hi,
    s=1: fp8 residual; partition p = channel p%64 with row-shift p//64;
    two-slot j selects tap k = p//64 + 2j (k=3 -> 0)."""
    w16 = wt.astype(np.float64).transpose(1, 2, 3, 0) * WSCALE  # [c,k,l,o]
    wk = np.zeros((C, 4, 3, O), np.float64)
    wk[:, 0:3] = w16
    hi = np.asarray(wk.astype(ml_dtypes.float8_e4m3), np.float64)
    lo = (wk - hi).astype(ml_dtypes.float8_e4m3)
    out = np.zeros((128, 768), ml_dtypes.float8_e4m3)
    for s, ww in ((0, hi.astype(ml_dtypes.float8_e4m3)), (1, lo)):
        for l in range(3):
            for h in range(2):
                for j in range(2):
                    out[64 * h : 64 * h + 64,
                        (3 * s + l) * 128 + 64 * j : (3 * s + l) * 128 + 64 * j + 64,
                        ] = np.asarray(ww)[:, h + 2 * j, l, :]
    return np.ascontiguousarray(out)


def kernel(inputs, alpha, weight, bias, a, b, c):
    x = np.asarray(inputs, np.float32)
    al = np.asarray(alpha, np.float32)
    wt = np.asarray(weight, np.float32)
    bs = np.asarray(bias, np.float32)
    av, bv, cv = float(a), float(b), float(c)

    if "nc" not in _cache:
        nc_new = _program()
        nc_new.finalize()
        _cache["nc"] = nc_new
    nc = _cache["nc"]

    w_packed = _pack_weights(wt)
    b_packed = np.ascontiguousarray(bs.reshape(O, 1))

    in_maps = []
    for core in range(8):
        b_idx, hh = divmod(core, 2)
        r0 = hh * HS - 1  # global row of padded row 0
        ys = np.zeros((C, NR + 1, WP2), np.float32)
        als = np.zeros((1, NR + 1, WP2), np.float32)
        lo_r = max(0, r0)
        hi_r = min(H, r0 + HS + 2)
        ys[:, lo_r - r0 : hi_r - r0, 1 : 1 + W] = x[b_idx, :, lo_r:hi_r, :]
        als[:, lo_r - r0 : hi_r - r0, 1 : 1 + W] = al[b_idx, :, lo_r:hi_r, :]
        y = ys * ((av * als + bv) * als + cv)  # y = x * p
        y_hi = y.astype(ml_dtypes.float8_e4m3)
        y_lo = (y - np.asarray(y_hi, np.float32)).astype(ml_dtypes.float8_e4m3)

        def pack(yv):
            return np.concatenate(
                [yv[:, 0:NR].reshape(C, FREE3), yv[:, 1 : NR + 1].reshape(C, FREE3)],
                axis=0,
            )

        in_maps.append(
            {
                "yh": np.ascontiguousarray(pack(y_hi)),
                "yl": np.ascontiguousarray(pack(y_lo)),
                "w": w_packed,
                "bias": b_packed,
            }
        )

    res = run_bass_kernel_spmd(nc, in_maps, list(range(8)))

    out = np.empty((B, O, H, W), np.float32)
    for core in range(8):
        b_idx, hh = divmod(core, 2)
        out[b_idx, :, hh * HS : (hh + 1) * HS, :] = res.results[core]["out"].reshape(
            O, HS, W
        )
    return out


# revision 43
# speedup vs baseline: 1.0161x; 1.0034x over previous
"""ConvSquare Trainium2 kernel (fp8 DoubleRow hi/lo formulation).

Math: out = conv2d_3x3(x * p, weight) + bias, stride 1, pad 1, where
p = (a*alpha + b)*alpha + c on the zero-padded alpha field.

Sharding: 8 cores = batch(4) x row-half(2); each core emits [64, 64, 128].

Device pipeline per core:
  - Host precomputes y = x*p (elementwise prep, 0.01% of FLOPs) and splits
    it into fp8e4m3 hi + lo residual streams, packed with a row pitch of
    136 so the DoubleRow Ko stride (2 rows = 272 elems) is 16-aligned.
    Partitions 0-63 hold rows 0..67, partitions 64-127 hold rows 1..68.
  - Weights are scaled x16 and split hi/lo in fp8; the ACT PSUM->SBUF
    copy unscales via its activation `scale` and adds bias.
  - One DoubleRow matmul contracts 4 taps at once: partition halves give
    row shifts {0,1}, the Ko pair dim (+272 elems = +2 rows) gives {2,3}
    (tap k=3 has zero weight). 3 DR matmuls cover the 9 taps per pass;
    3 passes (wh*yh, wh*yl, wl*yh) give ~bf16 accuracy at 0.5 cyc/row:
    9 DR matmuls x 128-free per output row = 576 PE cycles vs 768 bf16.
  - Small first/last row groups, grouped stores fanned across queues,
    two warm-up matmuls to open the PE clock-ramp window early.
"""

import sys

import numpy as np

sys.path.insert(0, "/opt/trn_rl_repo")

import ml_dtypes

import concourse.bass as bass
import concourse.mybir as mybir
from concourse.bass_utils import run_bass_kernel_spmd
from concourse.tile import TileContext

F32 = mybir.dt.float32
BF16 = mybir.dt.bfloat16
FP8 = mybir.dt.float8e4

B, C, O, H, W = 4, 64, 64, 128, 128
HS = 64  # output rows per core
WP2 = 136  # padded row pitch (130 data cols, padded so 2 rows % 16 == 0)
NR = 68  # rows per stored half-slab
FREE3 = NR * WP2  # 9248
KO = 2 * WP2  # 272: DoubleRow Ko stride (+2 rows)
WSCALE = 16.0
N_WARM = 2
# y-stream chunk edges in rows (both hi and lo streams)
ROW_EDGES = [0, 6, 12, 20, 28, 38, 48, 58, 68]
# matmul groups: (start_row, n_rows)
MM_CHUNKS = (
    [(0, 2), (2, 2), (4, 4)]
    + [(8 + 4 * i, 4) for i in range(13)]
    + [(62, 2), (60, 2)]
)
STORE_GROUPS = [
    (0, 3072),
    (3072, 5120),
    (5120, 6656),
    (6656, 7680),
    (7680, 7936),
]

_cache: dict = {}


def _program() -> bass.Bass:
    from concourse.bacc import Bacc

    nc = Bacc()
    yh_h = nc.dram_tensor("yh", [128, FREE3], FP8, kind="ExternalInput")
    yl_h = nc.dram_tensor("yl", [128, FREE3], FP8, kind="ExternalInput")
    # w8: blocks (s, l): cols (s*3+l)*128 + two*64 + o; s=0 hi, s=1 lo
    w_h = nc.dram_tensor("w", [128, 768], FP8, kind="ExternalInput")
    bias_h = nc.dram_tensor("bias", [O, 1], F32, kind="ExternalInput")
    out_h = nc.dram_tensor("out", [O, HS * W], F32, kind="ExternalOutput")

    with TileContext(nc) as tc:
        with (
            tc.tile_pool(name="const", bufs=1) as cpool,
            tc.tile_pool(name="work", bufs=1) as wpool,
            tc.tile_pool(name="psum", bufs=4, space="PSUM") as ppool,
        ):
            # PE warm-up to start the clock-ramp window early
            wrm_w = cpool.tile([1, 1], BF16)
            wrm_r = cpool.tile([1, 512], BF16)
            nc.vector.memset(wrm_w[:, :], 0.0)
            nc.vector.memset(wrm_r[:, :], 0.0)
            for _ in range(N_WARM):
                pw = ppool.tile([O, 512], F32)
                nc.tensor.matmul(
                    pw[0:1, :], wrm_w[:, :], wrm_r[:, :], start=True, stop=True
                )

            wt = cpool.tile([128, 768], FP8)
            bt = cpool.tile([O, 1], F32)
            yh = wpool.tile([128, FREE3], FP8)
            yl = wpool.tile([128, FREE3], FP8)
            st = wpool.tile([O, HS * W], F32)

            # weights/bias via Pool SWDGE (no HWDGE slot)
            nc.gpsimd.dma_start(out=wt[:, :], in_=w_h[:, :])
            nc.gpsimd.dma_start(out=bt[:, :], in_=bias_h[:, :])

            for j in range(len(ROW_EDGES) - 1):
                c0 = ROW_EDGES[j] * WP2
                c1 = ROW_EDGES[j + 1] * WP2
                nc.sync.dma_start(out=yh[:, c0:c1], in_=yh_h[:, c0:c1])
                nc.sync.dma_start(out=yl[:, c0:c1], in_=yl_h[:, c0:c1])

            # lhsT blocks: [p, block (s*3+l), two, o]
            w4 = wt[:].rearrange("p (b two o) -> p b two o", b=6, two=2)
            yh3 = yh[:].rearrange("p (r c) -> p r c", r=NR)
            yl3 = yl[:].rearrange("p (r c) -> p r c", r=NR)

            def rhs(stream3, m, l):
                # rows {m, m+2} via step-2 slice = the DoubleRow Ko pair
                return stream3[0:128, m : m + 3 : 2, l : l + W]

            # yl-dependent hl pass last within each row: the PE exec queue
            # is FIFO, so a late yl chunk must not block yh-only matmuls
            passes = ((0, yh3), (1, yh3), (0, yl3))
            for R, r in MM_CHUNKS:
                ps = ppool.tile([O, r * W], F32)
                for i in range(r):
                    m = R + i
                    sl = ps[:, W * i : W * (i + 1)]
                    n9 = 0
                    for s, ystream in passes:
                        for l in range(3):
                            nc.tensor.matmul(
                                sl,
                                w4[0:128, 3 * s + l, 0:2, 0:64],
                                rhs(ystream, m, l),
                                start=(n9 == 0),
                                stop=(n9 == 8),
                                perf_mode=mybir.MatmulPerfMode.DoubleRow,
                            )
                            n9 += 1
                # unscale (1/16) + bias while copying PSUM -> SBUF staging
                ss = st[:, W * R : W * (R + r)]
                if R == 62:
                    nc.vector.tensor_scalar(
                        out=ss, in0=ps[:, :], scalar1=1.0 / WSCALE,
                        scalar2=bt[:, 0:1], op0=mybir.AluOpType.mult,
                        op1=mybir.AluOpType.add,
                    )
                else:
                    nc.scalar.activation(
                        ss, ps[:, :], mybir.ActivationFunctionType.Identity,
                        bias=bt[:, 0:1], scale=1.0 / WSCALE,
                    )
            for g0, g1 in STORE_GROUPS:
                nc.sync.dma_start(out=out_h[:, g0:g1], in_=st[:, g0:g1])
            # tail stores on different queues so their issue overlaps
            nc.gpsimd.dma_start(out=out_h[:, 7936:8064], in_=st[:, 7936:8064])
            nc.scalar.dma_start(out=out_h[:, 8064:8192], in_=st[:, 8064:8192])
    return nc


def _pack_weights(wt):
    """[O,C,3,3] -> [128, 768] fp8: blocks (s,l), s=0: fp8(16w) hi,
    s=1: fp8 residual; partition p = channel p%64 with row-shift p//64;
    two-slot j selects tap k = p//64 + 2j (k=3 -> 0)."""
    w16 = wt.astype(np.float64).transpose(1, 2, 3, 0) * WSCALE  # [c,k,l,o]
    wk = np.zeros((C, 4, 3, O), np.float64)
    wk[:, 0:3] = w16
    hi = np.asarray(wk.astype(ml_dtypes.float8_e4m3), np.float64)
    lo = (wk - hi).astype(ml_dtypes.float8_e4m3)
    out = np.zeros((128, 768), ml_dtypes.float8_e4m3)
    for s, ww in ((0, hi.astype(ml_dtypes.float8_e4m3)), (1, lo)):
        for l in range(3):
            for h in range(2):
                for j in range(2):
                    out[64 * h : 64 * h + 64,
                        (3 * s + l) * 128 + 64 * j : (3 * s + l) * 128 + 64 * j + 64,
                        ] = np.asarray(ww)[:, h + 2 * j, l, :]
    return np.ascontiguousarray(out)


def kernel(inputs, alpha, weight, bias, a, b, c):
    x = np.asarray(inputs, np.float32)
    al = np.asarray(alpha, np.float32)
    wt = np.asarray(weight, np.float32)
    bs = np.asarray(bias, np.float32)
    av, bv, cv = float(a), float(b), float(c)

    if "nc" not in _cache:
        nc_new = _program()
        nc_new.finalize()
        _cache["nc"] = nc_new
    nc = _cache["nc"]

    w_packed = _pack_weights(wt)
    b_packed = np.ascontiguousarray(bs.reshape(O, 1))

    in_maps = []
    for core in range(8):
        b_idx, hh = divmod(core, 2)
        r0 = hh * HS - 1  # global row of padded row 0
        ys = np.zeros((C, NR + 1, WP2), np.float32)
        als = np.zeros((1, NR + 1, WP2), np.float32)
        lo_r = max(0, r0)
        hi_r = min(H, r0 + HS + 2)
        ys[:, lo_r - r0 : hi_r - r0, 1 : 1 + W] = x[b_idx, :, lo_r:hi_r, :]
        als[:, lo_r - r0 : hi_r - r0, 1 : 1 + W] = al[b_idx, :, lo_r:hi_r, :]
        y = ys * ((av * als + bv) * als + cv)  # y = x * p
        y_hi = y.astype(ml_dtypes.float8_e4m3)
        y_lo = (y - np.asarray(y_hi, np.float32)).astype(ml_dtypes.float8_e4m3)

        def pack(yv):
            return np.concatenate(
                [yv[:, 0:NR].reshape(C, FREE3), yv[:, 1 : NR + 1].reshape(C, FREE3)],
                axis=0,
            )

        in_maps.append(
            {
                "yh": np.ascontiguousarray(pack(y_hi)),
                "yl": np.ascontiguousarray(pack(y_lo)),
                "w": w_packed,
                "bias": b_packed,
            }
        )

    res = run_bass_kernel_spmd(nc, in_maps, list(range(8)))

    out = np.empty((B, O, H, W), np.float32)
    for core in range(8):
        b_idx, hh = divmod(core, 2)
        out[b_idx, :, hh * HS : (hh + 1) * HS, :] = res.results[core]["out"].reshape(
            O, HS, W
        )
    return out


# revision 44
# speedup vs baseline: 1.0257x; 1.0094x over previous
"""ConvSquare Trainium2 kernel (fp8 DoubleRow hi/lo formulation).

Math: out = conv2d_3x3(x * p, weight) + bias, stride 1, pad 1, where
p = (a*alpha + b)*alpha + c on the zero-padded alpha field.

Sharding: 8 cores = batch(4) x row-half(2); each core emits [64, 64, 128].

Device pipeline per core:
  - Host precomputes y = x*p (elementwise prep, 0.01% of FLOPs) and splits
    it into fp8e4m3 hi + lo residual streams, packed with a row pitch of
    136 so the DoubleRow Ko stride (2 rows = 272 elems) is 16-aligned.
    Partitions 0-63 hold rows 0..67, partitions 64-127 hold rows 1..68.
  - Weights are scaled x16 and split hi/lo in fp8; the ACT PSUM->SBUF
    copy unscales via its activation `scale` and adds bias.
  - One DoubleRow matmul contracts 4 taps at once: partition halves give
    row shifts {0,1}, the Ko pair dim (+272 elems = +2 rows) gives {2,3}
    (tap k=3 has zero weight). 3 DR matmuls cover the 9 taps per pass;
    3 passes (wh*yh, wh*yl, wl*yh) give ~bf16 accuracy at 0.5 cyc/row:
    9 DR matmuls x 128-free per output row = 576 PE cycles vs 768 bf16.
  - Small first/last row groups, grouped stores fanned across queues,
    two warm-up matmuls to open the PE clock-ramp window early.
"""

import sys

import numpy as np

sys.path.insert(0, "/opt/trn_rl_repo")

import ml_dtypes

import concourse.bass as bass
import concourse.mybir as mybir
from concourse.bass_utils import run_bass_kernel_spmd
from concourse.tile import TileContext

F32 = mybir.dt.float32
BF16 = mybir.dt.bfloat16
FP8 = mybir.dt.float8e4

B, C, O, H, W = 4, 64, 64, 128, 128
HS = 64  # output rows per core
WP2 = 136  # padded row pitch (130 data cols, padded so 2 rows % 16 == 0)
NR = 68  # rows per stored half-slab
FREE3 = NR * WP2  # 9248
KO = 2 * WP2  # 272: DoubleRow Ko stride (+2 rows)
WSCALE = 16.0
N_WARM = 2
# y-stream chunk edges in rows (both hi and lo streams)
ROW_EDGES = [0, 6, 12, 20, 28, 38, 48, 58, 68]
# matmul groups: (start_row, n_rows)
MM_CHUNKS = (
    [(0, 2), (2, 2), (4, 4)]
    + [(8 + 4 * i, 4) for i in range(13)]
    + [(62, 2), (60, 2)]
)
STORE_GROUPS = [
    (0, 3072),
    (3072, 5120),
    (5120, 6656),
    (6656, 7680),
    (7680, 7936),
]

_cache: dict = {}


def _program() -> bass.Bass:
    from concourse.bacc import Bacc

    nc = Bacc()
    yh_h = nc.dram_tensor("yh", [128, FREE3], FP8, kind="ExternalInput")
    yl_h = nc.dram_tensor("yl", [128, FREE3], FP8, kind="ExternalInput")
    # w8: blocks (s, l): cols (s*3+l)*128 + two*64 + o; s=0 hi, s=1 lo
    w_h = nc.dram_tensor("w", [128, 768], FP8, kind="ExternalInput")
    bias_h = nc.dram_tensor("bias", [O, 1], F32, kind="ExternalInput")
    out_h = nc.dram_tensor("out", [O, HS * W], F32, kind="ExternalOutput")

    with TileContext(nc) as tc:
        with (
            tc.tile_pool(name="const", bufs=1) as cpool,
            tc.tile_pool(name="work", bufs=1) as wpool,
            tc.tile_pool(name="psum", bufs=4, space="PSUM") as ppool,
        ):
            # PE warm-up to start the clock-ramp window early
            wrm_w = cpool.tile([1, 1], BF16)
            wrm_r = cpool.tile([1, 512], BF16)
            nc.vector.memset(wrm_w[:, :], 0.0)
            nc.vector.memset(wrm_r[:, :], 0.0)
            for _ in range(N_WARM):
                pw = ppool.tile([O, 512], F32)
                nc.tensor.matmul(
                    pw[0:1, :], wrm_w[:, :], wrm_r[:, :], start=True, stop=True
                )

            wt = cpool.tile([128, 768], FP8)
            bt = cpool.tile([O, 1], F32)
            yh = wpool.tile([128, FREE3], FP8)
            yl = wpool.tile([128, FREE3], FP8)
            st = wpool.tile([O, HS * W], F32)

            # weights/bias via Pool SWDGE (no HWDGE slot)
            nc.gpsimd.dma_start(out=wt[:, :], in_=w_h[:, :])
            nc.gpsimd.dma_start(out=bt[:, :], in_=bias_h[:, :])

            for j in range(len(ROW_EDGES) - 1):
                c0 = ROW_EDGES[j] * WP2
                c1 = ROW_EDGES[j + 1] * WP2
                nc.sync.dma_start(out=yh[:, c0:c1], in_=yh_h[:, c0:c1])
                nc.sync.dma_start(out=yl[:, c0:c1], in_=yl_h[:, c0:c1])

            # lhsT blocks: [p, block (s*3+l), two, o]
            w4 = wt[:].rearrange("p (b two o) -> p b two o", b=6, two=2)
            yh3 = yh[:].rearrange("p (r c) -> p r c", r=NR)
            yl3 = yl[:].rearrange("p (r c) -> p r c", r=NR)

            def rhs(stream3, m, l):
                # rows {m, m+2} via step-2 slice = the DoubleRow Ko pair
                return stream3[0:128, m : m + 3 : 2, l : l + W]

            # yl-dependent hl pass last within each row: the PE exec queue
            # is FIFO, so a late yl chunk must not block yh-only matmuls
            # pass-major emission: all yh-dependent matmuls (hh, lh) of the
            # whole group run before the yl-dependent hl pass, so a late yl
            # chunk never blocks yh-only work in the FIFO PE queue. PSUM
            # start=True zeroes the whole 2KB bank (lazily, per first write),
            # so exactly ONE start for the tile; later slices accumulate
            # from the pending-zero state with start=False.
            passes = ((0, yh3), (1, yh3), (0, yl3))
            for R, r in MM_CHUNKS:
                ps = ppool.tile([O, r * W], F32)
                first = True
                for pi, (s, ystream) in enumerate(passes):
                    for i in range(r):
                        m = R + i
                        sl = ps[:, W * i : W * (i + 1)]
                        for l in range(3):
                            nc.tensor.matmul(
                                sl,
                                w4[0:128, 3 * s + l, 0:2, 0:64],
                                rhs(ystream, m, l),
                                start=first,
                                stop=(pi == 2 and i == r - 1 and l == 2),
                                perf_mode=mybir.MatmulPerfMode.DoubleRow,
                                skip_group_check=True,
                            )
                            first = False
                # unscale (1/16) + bias while copying PSUM -> SBUF staging
                ss = st[:, W * R : W * (R + r)]
                if R == 62:
                    nc.vector.tensor_scalar(
                        out=ss, in0=ps[:, :], scalar1=1.0 / WSCALE,
                        scalar2=bt[:, 0:1], op0=mybir.AluOpType.mult,
                        op1=mybir.AluOpType.add,
                    )
                else:
                    nc.scalar.activation(
                        ss, ps[:, :], mybir.ActivationFunctionType.Identity,
                        bias=bt[:, 0:1], scale=1.0 / WSCALE,
                    )
            for g0, g1 in STORE_GROUPS:
                nc.sync.dma_start(out=out_h[:, g0:g1], in_=st[:, g0:g1])
            # tail stores on different queues so their issue overlaps
            nc.gpsimd.dma_start(out=out_h[:, 7936:8064], in_=st[:, 7936:8064])
            nc.scalar.dma_start(out=out_h[:, 8064:8192], in_=st[:, 8064:8192])
    return nc


def _pack_weights(wt):
    """[O,C,3,3] -> [128, 768] fp8: blocks (s,l), s=0: fp8(16w) hi,
    s=1: fp8 residual; partition p = channel p%64 with row-shift p//64;
    two-slot j selects tap k = p//64 + 2j (k=3 -> 0)."""
    w16 = wt.astype(np.float64).transpose(1, 2, 3, 0) * WSCALE  # [c,k,l,o]
    wk = np.zeros((C, 4, 3, O), np.float64)
    wk[:, 0:3] = w16
    hi = np.asarray(wk.astype(ml_dtypes.float8_e4m3), np.float64)
    lo = (wk - hi).astype(ml_dtypes.float8_e4m3)
    out = np.zeros((128, 768), ml_dtypes.float8_e4m3)
    for s, ww in ((0, hi.astype(ml_dtypes.float8_e4m3)), (1, lo)):
        for l in range(3):
            for h in range(2):
                for j in range(2):
                    out[64 * h : 64 * h + 64,
                        (3 * s + l) * 128 + 64 * j : (3 * s + l) * 128 + 64 * j + 64,
                        ] = np.asarray(ww)[:, h + 2 * j, l, :]
    return np.ascontiguousarray(out)


def kernel(inputs, alpha, weight, bias, a, b, c):
    x = np.asarray(inputs, np.float32)
    al = np.asarray(alpha, np.float32)
    wt = np.asarray(weight, np.float32)
    bs = np.asarray(bias, np.float32)
    av, bv, cv = float(a), float(b), float(c)

    if "nc" not in _cache:
        nc_new = _program()
        nc_new.finalize()
        _cache["nc"] = nc_new
    nc = _cache["nc"]

    w_packed = _pack_weights(wt)
    b_packed = np.ascontiguousarray(bs.reshape(O, 1))

    in_maps = []
    for core in range(8):
        b_idx, hh = divmod(core, 2)
        r0 = hh * HS - 1  # global row of padded row 0
        ys = np.zeros((C, NR + 1, WP2), np.float32)
        als = np.zeros((1, NR + 1, WP2), np.float32)
        lo_r = max(0, r0)
        hi_r = min(H, r0 + HS + 2)
        ys[:, lo_r - r0 : hi_r - r0, 1 : 1 + W] = x[b_idx, :, lo_r:hi_r, :]
        als[:, lo_r - r0 : hi_r - r0, 1 : 1 + W] = al[b_idx, :, lo_r:hi_r, :]
        y = ys * ((av * als + bv) * als + cv)  # y = x * p
        y_hi = y.astype(ml_dtypes.float8_e4m3)
        y_lo = (y - np.asarray(y_hi, np.float32)).astype(ml_dtypes.float8_e4m3)

        def pack(yv):
            return np.concatenate(
                [yv[:, 0:NR].reshape(C, FREE3), yv[:, 1 : NR + 1].reshape(C, FREE3)],
                axis=0,
            )

        in_maps.append(
            {
                "yh": np.ascontiguousarray(pack(y_hi)),
                "yl": np.ascontiguousarray(pack(y_lo)),
                "w": w_packed,
                "bias": b_packed,
            }
        )

    res = run_bass_kernel_spmd(nc, in_maps, list(range(8)))

    out = np.empty((B, O, H, W), np.float32)
    for core in range(8):
        b_idx, hh = divmod(core, 2)
        out[b_idx, :, hh * HS : (hh + 1) * HS, :] = res.results[core]["out"].reshape(
            O, HS, W
        )
    return out


# revision 45
# speedup vs baseline: 1.0304x; 1.0045x over previous
"""ConvSquare Trainium2 kernel (fp8 DoubleRow hi/lo formulation).

Math: out = conv2d_3x3(x * p, weight) + bias, stride 1, pad 1, where
p = (a*alpha + b)*alpha + c on the zero-padded alpha field.

Sharding: 8 cores = batch(4) x row-half(2); each core emits [64, 64, 128].

Device pipeline per core:
  - Host precomputes y = x*p (elementwise prep, 0.01% of FLOPs) and splits
    it into fp8e4m3 hi + lo residual streams, packed with a row pitch of
    136 so the DoubleRow Ko stride (2 rows = 272 elems) is 16-aligned.
    Partitions 0-63 hold rows 0..67, partitions 64-127 hold rows 1..68.
  - Weights are scaled x16 and split hi/lo in fp8; the ACT PSUM->SBUF
    copy unscales via its activation `scale` and adds bias.
  - One DoubleRow matmul contracts 4 taps at once: partition halves give
    row shifts {0,1}, the Ko pair dim (+272 elems = +2 rows) gives {2,3}
    (tap k=3 has zero weight). 3 DR matmuls cover the 9 taps per pass;
    3 passes (wh*yh, wh*yl, wl*yh) give ~bf16 accuracy at 0.5 cyc/row:
    9 DR matmuls x 128-free per output row = 576 PE cycles vs 768 bf16.
  - Small first/last row groups, grouped stores fanned across queues,
    two warm-up matmuls to open the PE clock-ramp window early.
"""

import sys

import numpy as np

sys.path.insert(0, "/opt/trn_rl_repo")

import ml_dtypes

import concourse.bass as bass
import concourse.mybir as mybir
from concourse.bass_utils import run_bass_kernel_spmd
from concourse.tile import TileContext

F32 = mybir.dt.float32
BF16 = mybir.dt.bfloat16
FP8 = mybir.dt.float8e4

B, C, O, H, W = 4, 64, 64, 128, 128
HS = 64  # output rows per core
WP2 = 136  # padded row pitch (130 data cols, padded so 2 rows % 16 == 0)
NR = 68  # rows per stored half-slab
FREE3 = NR * WP2  # 9248
KO = 2 * WP2  # 272: DoubleRow Ko stride (+2 rows)
WSCALE = 16.0
N_WARM = 2
# y-stream chunk edges in rows (both hi and lo streams)
ROW_EDGES = [0, 6, 14, 22, 32, 42, 52, 62, 68]
# matmul groups: (start_row, n_rows)
MM_CHUNKS = (
    [(0, 2), (2, 2), (4, 4)]
    + [(8 + 4 * i, 4) for i in range(13)]
    + [(62, 2), (60, 2)]
)
STORE_GROUPS = [
    (0, 3072),
    (3072, 5120),
    (5120, 6656),
    (6656, 7680),
    (7680, 7936),
]

_cache: dict = {}


def _program() -> bass.Bass:
    from concourse.bacc import Bacc

    nc = Bacc()
    yh_h = nc.dram_tensor("yh", [128, FREE3], FP8, kind="ExternalInput")
    yl_h = nc.dram_tensor("yl", [128, FREE3], FP8, kind="ExternalInput")
    # w8: blocks (s, l): cols (s*3+l)*128 + two*64 + o; s=0 hi, s=1 lo
    w_h = nc.dram_tensor("w", [128, 768], FP8, kind="ExternalInput")
    bias_h = nc.dram_tensor("bias", [O, 1], F32, kind="ExternalInput")
    out_h = nc.dram_tensor("out", [O, HS * W], F32, kind="ExternalOutput")

    with TileContext(nc) as tc:
        with (
            tc.tile_pool(name="const", bufs=1) as cpool,
            tc.tile_pool(name="work", bufs=1) as wpool,
            tc.tile_pool(name="psum", bufs=4, space="PSUM") as ppool,
        ):
            # PE warm-up to start the clock-ramp window early
            wrm_w = cpool.tile([1, 1], BF16)
            wrm_r = cpool.tile([1, 512], BF16)
            nc.vector.memset(wrm_w[:, :], 0.0)
            nc.vector.memset(wrm_r[:, :], 0.0)
            for _ in range(N_WARM):
                pw = ppool.tile([O, 512], F32)
                nc.tensor.matmul(
                    pw[0:1, :], wrm_w[:, :], wrm_r[:, :], start=True, stop=True
                )

            wt = cpool.tile([128, 768], FP8)
            bt = cpool.tile([O, 1], F32)
            yh = wpool.tile([128, FREE3], FP8)
            yl = wpool.tile([128, FREE3], FP8)
            st = wpool.tile([O, HS * W], F32)

            # weights/bias via Pool SWDGE (no HWDGE slot)
            nc.gpsimd.dma_start(out=wt[:, :], in_=w_h[:, :])
            nc.gpsimd.dma_start(out=bt[:, :], in_=bias_h[:, :])

            for j in range(len(ROW_EDGES) - 1):
                c0 = ROW_EDGES[j] * WP2
                c1 = ROW_EDGES[j + 1] * WP2
                nc.sync.dma_start(out=yh[:, c0:c1], in_=yh_h[:, c0:c1])
                nc.sync.dma_start(out=yl[:, c0:c1], in_=yl_h[:, c0:c1])

            # lhsT blocks: [p, block (s*3+l), two, o]
            w4 = wt[:].rearrange("p (b two o) -> p b two o", b=6, two=2)
            yh3 = yh[:].rearrange("p (r c) -> p r c", r=NR)
            yl3 = yl[:].rearrange("p (r c) -> p r c", r=NR)

            def rhs(stream3, m, l):
                # rows {m, m+2} via step-2 slice = the DoubleRow Ko pair
                return stream3[0:128, m : m + 3 : 2, l : l + W]

            # yl-dependent hl pass last within each row: the PE exec queue
            # is FIFO, so a late yl chunk must not block yh-only matmuls
            # pass-major emission: all yh-dependent matmuls (hh, lh) of the
            # whole group run before the yl-dependent hl pass, so a late yl
            # chunk never blocks yh-only work in the FIFO PE queue. PSUM
            # start=True zeroes the whole 2KB bank (lazily, per first write),
            # so exactly ONE start for the tile; later slices accumulate
            # from the pending-zero state with start=False.
            passes = ((0, yh3), (1, yh3), (0, yl3))
            for R, r in MM_CHUNKS:
                ps = ppool.tile([O, r * W], F32)
                first = True
                for pi, (s, ystream) in enumerate(passes):
                    for i in range(r):
                        m = R + i
                        sl = ps[:, W * i : W * (i + 1)]
                        for l in range(3):
                            nc.tensor.matmul(
                                sl,
                                w4[0:128, 3 * s + l, 0:2, 0:64],
                                rhs(ystream, m, l),
                                start=first,
                                stop=(pi == 2 and i == r - 1 and l == 2),
                                perf_mode=mybir.MatmulPerfMode.DoubleRow,
                                skip_group_check=True,
                            )
                            first = False
                # unscale (1/16) + bias while copying PSUM -> SBUF staging
                ss = st[:, W * R : W * (R + r)]
                if R == 62:
                    nc.vector.tensor_scalar(
                        out=ss, in0=ps[:, :], scalar1=1.0 / WSCALE,
                        scalar2=bt[:, 0:1], op0=mybir.AluOpType.mult,
                        op1=mybir.AluOpType.add,
                    )
                else:
                    nc.scalar.activation(
                        ss, ps[:, :], mybir.ActivationFunctionType.Identity,
                        bias=bt[:, 0:1], scale=1.0 / WSCALE,
                    )
            for g0, g1 in STORE_GROUPS:
                nc.sync.dma_start(out=out_h[:, g0:g1], in_=st[:, g0:g1])
            # tail stores on different queues so their issue overlaps
            nc.gpsimd.dma_start(out=out_h[:, 7936:8064], in_=st[:, 7936:8064])
            nc.scalar.dma_start(out=out_h[:, 8064:8192], in_=st[:, 8064:8192])
    return nc


def _pack_weights(wt):
    """[O,C,3,3] -> [128, 768] fp8: blocks (s,l), s=0: fp8(16w) hi,
    s=1: fp8 residual; partition p = channel p%64 with row-shift p//64;
    two-slot j selects tap k = p//64 + 2j (k=3 -> 0)."""
    w16 = wt.astype(np.float64).transpose(1, 2, 3, 0) * WSCALE  # [c,k,l,o]
    wk = np.zeros((C, 4, 3, O), np.float64)
    wk[:, 0:3] = w16
    hi = np.asarray(wk.astype(ml_dtypes.float8_e4m3), np.float64)
    lo = (wk - hi).astype(ml_dtypes.float8_e4m3)
    out = np.zeros((128, 768), ml_dtypes.float8_e4m3)
    for s, ww in ((0, hi.astype(ml_dtypes.float8_e4m3)), (1, lo)):
        for l in range(3):
            for h in range(2):
                for j in range(2):
                    out[64 * h : 64 * h + 64,
                        (3 * s + l) * 128 + 64 * j : (3 * s + l) * 128 + 64 * j + 64,
                        ] = np.asarray(ww)[:, h + 2 * j, l, :]
    return np.ascontiguousarray(out)


def kernel(inputs, alpha, weight, bias, a, b, c):
    x = np.asarray(inputs, np.float32)
    al = np.asarray(alpha, np.float32)
    wt = np.asarray(weight, np.float32)
    bs = np.asarray(bias, np.float32)
    av, bv, cv = float(a), float(b), float(c)

    if "nc" not in _cache:
        nc_new = _program()
        nc_new.finalize()
        _cache["nc"] = nc_new
    nc = _cache["nc"]

    w_packed = _pack_weights(wt)
    b_packed = np.ascontiguousarray(bs.reshape(O, 1))

    in_maps = []
    for core in range(8):
        b_idx, hh = divmod(core, 2)
        r0 = hh * HS - 1  # global row of padded row 0
        ys = np.zeros((C, NR + 1, WP2), np.float32)
        als = np.zeros((1, NR + 1, WP2), np.float32)
        lo_r = max(0, r0)
        hi_r = min(H, r0 + HS + 2)
        ys[:, lo_r - r0 : hi_r - r0, 1 : 1 + W] = x[b_idx, :, lo_r:hi_r, :]
        als[:, lo_r - r0 : hi_r - r0, 1 : 1 + W] = al[b_idx, :, lo_r:hi_r, :]
        y = ys * ((av * als + bv) * als + cv)  # y = x * p
        y_hi = y.astype(ml_dtypes.float8_e4m3)
        y_lo = (y - np.asarray(y_hi, np.float32)).astype(ml_dtypes.float8_e4m3)

        def pack(yv):
            return np.concatenate(
                [yv[:, 0:NR].reshape(C, FREE3), yv[:, 1 : NR + 1].reshape(C, FREE3)],
                axis=0,
            )

        in_maps.append(
            {
                "yh": np.ascontiguousarray(pack(y_hi)),
                "yl": np.ascontiguousarray(pack(y_lo)),
                "w": w_packed,
                "bias": b_packed,
            }
        )

    res = run_bass_kernel_spmd(nc, in_maps, list(range(8)))

    out = np.empty((B, O, H, W), np.float32)
    for core in range(8):
        b_idx, hh = divmod(core, 2)
        out[b_idx, :, hh * HS : (hh + 1) * HS, :] = res.results[core]["out"].reshape(
            O, HS, W
        )
    return out


# revision 46
# speedup vs baseline: 1.0323x; 1.0019x over previous
"""ConvSquare Trainium2 kernel (fp8 DoubleRow hi/lo formulation).

Math: out = conv2d_3x3(x * p, weight) + bias, stride 1, pad 1, where
p = (a*alpha + b)*alpha + c on the zero-padded alpha field.

Sharding: 8 cores = batch(4) x row-half(2); each core emits [64, 64, 128].

Device pipeline per core:
  - Host precomputes y = x*p (elementwise prep, 0.01% of FLOPs) and splits
    it into fp8e4m3 hi + lo residual streams, packed with a row pitch of
    136 so the DoubleRow Ko stride (2 rows = 272 elems) is 16-aligned.
    Partitions 0-63 hold rows 0..67, partitions 64-127 hold rows 1..68.
  - Weights are scaled x16 and split hi/lo in fp8; the ACT PSUM->SBUF
    copy unscales via its activation `scale` and adds bias.
  - One DoubleRow matmul contracts 4 taps at once: partition halves give
    row shifts {0,1}, the Ko pair dim (+272 elems = +2 rows) gives {2,3}
    (tap k=3 has zero weight). 3 DR matmuls cover the 9 taps per pass;
    3 passes (wh*yh, wh*yl, wl*yh) give ~bf16 accuracy at 0.5 cyc/row:
    9 DR matmuls x 128-free per output row = 576 PE cycles vs 768 bf16.
  - Small first/last row groups, grouped stores fanned across queues,
    two warm-up matmuls to open the PE clock-ramp window early.
"""

import sys

import numpy as np

sys.path.insert(0, "/opt/trn_rl_repo")

import ml_dtypes

import concourse.bass as bass
import concourse.mybir as mybir
from concourse.bass_utils import run_bass_kernel_spmd
from concourse.tile import TileContext

F32 = mybir.dt.float32
BF16 = mybir.dt.bfloat16
FP8 = mybir.dt.float8e4

B, C, O, H, W = 4, 64, 64, 128, 128
HS = 64  # output rows per core
WP2 = 136  # padded row pitch (130 data cols, padded so 2 rows % 16 == 0)
NR = 68  # rows per stored half-slab
FREE3 = NR * WP2  # 9248
KO = 2 * WP2  # 272: DoubleRow Ko stride (+2 rows)
WSCALE = 16.0
N_WARM = 2
# y-stream chunk edges in rows (both hi and lo streams)
ROW_EDGES = [0, 6, 14, 22, 32, 42, 52, 62, 68]
# matmul groups: (start_row, n_rows)
MM_CHUNKS = (
    [(0, 4), (4, 4)]
    + [(8 + 4 * i, 4) for i in range(13)]
    + [(62, 2), (60, 2)]
)
STORE_GROUPS = [
    (0, 3072),
    (3072, 5120),
    (5120, 6656),
    (6656, 7680),
    (7680, 7936),
]

_cache: dict = {}


def _program() -> bass.Bass:
    from concourse.bacc import Bacc

    nc = Bacc()
    yh_h = nc.dram_tensor("yh", [128, FREE3], FP8, kind="ExternalInput")
    yl_h = nc.dram_tensor("yl", [128, FREE3], FP8, kind="ExternalInput")
    # w8: blocks (s, l): cols (s*3+l)*128 + two*64 + o; s=0 hi, s=1 lo
    w_h = nc.dram_tensor("w", [128, 768], FP8, kind="ExternalInput")
    bias_h = nc.dram_tensor("bias", [O, 1], F32, kind="ExternalInput")
    out_h = nc.dram_tensor("out", [O, HS * W], F32, kind="ExternalOutput")

    with TileContext(nc) as tc:
        with (
            tc.tile_pool(name="const", bufs=1) as cpool,
            tc.tile_pool(name="work", bufs=1) as wpool,
            tc.tile_pool(name="psum", bufs=4, space="PSUM") as ppool,
        ):
            # PE warm-up to start the clock-ramp window early
            wrm_w = cpool.tile([1, 1], BF16)
            wrm_r = cpool.tile([1, 512], BF16)
            nc.vector.memset(wrm_w[:, :], 0.0)
            nc.vector.memset(wrm_r[:, :], 0.0)
            for _ in range(N_WARM):
                pw = ppool.tile([O, 512], F32)
                nc.tensor.matmul(
                    pw[0:1, :], wrm_w[:, :], wrm_r[:, :], start=True, stop=True
                )

            wt = cpool.tile([128, 768], FP8)
            bt = cpool.tile([O, 1], F32)
            yh = wpool.tile([128, FREE3], FP8)
            yl = wpool.tile([128, FREE3], FP8)
            st = wpool.tile([O, HS * W], F32)

            # weights/bias via Pool SWDGE (no HWDGE slot)
            nc.gpsimd.dma_start(out=wt[:, :], in_=w_h[:, :])
            nc.gpsimd.dma_start(out=bt[:, :], in_=bias_h[:, :])

            for j in range(len(ROW_EDGES) - 1):
                c0 = ROW_EDGES[j] * WP2
                c1 = ROW_EDGES[j + 1] * WP2
                nc.sync.dma_start(out=yh[:, c0:c1], in_=yh_h[:, c0:c1])
                nc.sync.dma_start(out=yl[:, c0:c1], in_=yl_h[:, c0:c1])

            # lhsT blocks: [p, block (s*3+l), two, o]
            w4 = wt[:].rearrange("p (b two o) -> p b two o", b=6, two=2)
            yh3 = yh[:].rearrange("p (r c) -> p r c", r=NR)
            yl3 = yl[:].rearrange("p (r c) -> p r c", r=NR)

            def rhs(stream3, m, l):
                # rows {m, m+2} via step-2 slice = the DoubleRow Ko pair
                return stream3[0:128, m : m + 3 : 2, l : l + W]

            # yl-dependent hl pass last within each row: the PE exec queue
            # is FIFO, so a late yl chunk must not block yh-only matmuls
            # pass-major emission: all yh-dependent matmuls (hh, lh) of the
            # whole group run before the yl-dependent hl pass, so a late yl
            # chunk never blocks yh-only work in the FIFO PE queue. PSUM
            # start=True zeroes the whole 2KB bank (lazily, per first write),
            # so exactly ONE start for the tile; later slices accumulate
            # from the pending-zero state with start=False.
            passes = ((0, yh3), (1, yh3), (0, yl3))
            for R, r in MM_CHUNKS:
                ps = ppool.tile([O, r * W], F32)
                first = True
                for pi, (s, ystream) in enumerate(passes):
                    for i in range(r):
                        m = R + i
                        sl = ps[:, W * i : W * (i + 1)]
                        for l in range(3):
                            nc.tensor.matmul(
                                sl,
                                w4[0:128, 3 * s + l, 0:2, 0:64],
                                rhs(ystream, m, l),
                                start=first,
                                stop=(pi == 2 and i == r - 1 and l == 2),
                                perf_mode=mybir.MatmulPerfMode.DoubleRow,
                                skip_group_check=True,
                            )
                            first = False
                # unscale (1/16) + bias while copying PSUM -> SBUF staging
                ss = st[:, W * R : W * (R + r)]
                if R == 62:
                    nc.vector.tensor_scalar(
                        out=ss, in0=ps[:, :], scalar1=1.0 / WSCALE,
                        scalar2=bt[:, 0:1], op0=mybir.AluOpType.mult,
                        op1=mybir.AluOpType.add,
                    )
                else:
                    nc.scalar.activation(
                        ss, ps[:, :], mybir.ActivationFunctionType.Identity,
                        bias=bt[:, 0:1], scale=1.0 / WSCALE,
                    )
            for g0, g1 in STORE_GROUPS:
                nc.sync.dma_start(out=out_h[:, g0:g1], in_=st[:, g0:g1])
            # tail stores on different queues so their issue overlaps
            nc.gpsimd.dma_start(out=out_h[:, 7936:8064], in_=st[:, 7936:8064])
            nc.scalar.dma_start(out=out_h[:, 8064:8192], in_=st[:, 8064:8192])
    return nc


def _pack_weights(wt):
    """[O,C,3,3] -> [128, 768] fp8: blocks (s,l), s=0: fp8(16w) hi,
    s=1: fp8 residual; partition p = channel p%64 with row-shift p//64;
    two-slot j selects tap k = p//64 + 2j (k=3 -> 0)."""
    w16 = wt.astype(np.float64).transpose(1, 2, 3, 0) * WSCALE  # [c,k,l,o]
    wk = np.zeros((C, 4, 3, O), np.float64)
    wk[:, 0:3] = w16
    hi = np.asarray(wk.astype(ml_dtypes.float8_e4m3), np.float64)
    lo = (wk - hi).astype(ml_dtypes.float8_e4m3)
    out = np.zeros((128, 768), ml_dtypes.float8_e4m3)
    for s, ww in ((0, hi.astype(ml_dtypes.float8_e4m3)), (1, lo)):
        for l in range(3):
            for h in range(2):
                for j in range(2):
                    out[64 * h : 64 * h + 64,
                        (3 * s + l) * 128 + 64 * j : (3 * s + l) * 128 + 64 * j + 64,
                        ] = np.asarray(ww)[:, h + 2 * j, l, :]
    return np.ascontiguousarray(out)


def kernel(inputs, alpha, weight, bias, a, b, c):
    x = np.asarray(inputs, np.float32)
    al = np.asarray(alpha, np.float32)
    wt = np.asarray(weight, np.float32)
    bs = np.asarray(bias, np.float32)
    av, bv, cv = float(a), float(b), float(c)

    if "nc" not in _cache:
        nc_new = _program()
        nc_new.finalize()
        _cache["nc"] = nc_new
    nc = _cache["nc"]

    w_packed = _pack_weights(wt)
    b_packed = np.ascontiguousarray(bs.reshape(O, 1))

    in_maps = []
    for core in range(8):
        b_idx, hh = divmod(core, 2)
        r0 = hh * HS - 1  # global row of padded row 0
        ys = np.zeros((C, NR + 1, WP2), np.float32)
        als = np.zeros((1, NR + 1, WP2), np.float32)
        lo_r = max(0, r0)
        hi_r = min(H, r0 + HS + 2)
        ys[:, lo_r - r0 : hi_r - r0, 1 : 1 + W] = x[b_idx, :, lo_r:hi_r, :]
        als[:, lo_r - r0 : hi_r - r0, 1 : 1 + W] = al[b_idx, :, lo_r:hi_r, :]
        y = ys * ((av * als + bv) * als + cv)  # y = x * p
        y_hi = y.astype(ml_dtypes.float8_e4m3)
        y_lo = (y - np.asarray(y_hi, np.float32)).astype(ml_dtypes.float8_e4m3)

        def pack(yv):
            return np.concatenate(
                [yv[:, 0:NR].reshape(C, FREE3), yv[:, 1 : NR + 1].reshape(C, FREE3)],
                axis=0,
            )

        in_maps.append(
            {
                "yh": np.ascontiguousarray(pack(y_hi)),
                "yl": np.ascontiguousarray(pack(y_lo)),
                "w": w_packed,
                "bias": b_packed,
            }
        )

    res = run_bass_kernel_spmd(nc, in_maps, list(range(8)))

    out = np.empty((B, O, H, W), np.float32)
    for core in range(8):
        b_idx, hh = divmod(core, 2)
        out[b_idx, :, hh * HS : (hh + 1) * HS, :] = res.results[core]["out"].reshape(
            O, HS, W
        )
    return out


# revision 47
# speedup vs baseline: 1.0345x; 1.0021x over previous
"""ConvSquare Trainium2 kernel (fp8 DoubleRow hi/lo formulation).

Math: out = conv2d_3x3(x * p, weight) + bias, stride 1, pad 1, where
p = (a*alpha + b)*alpha + c on the zero-padded alpha field.

Sharding: 8 cores = batch(4) x row-half(2); each core emits [64, 64, 128].

Device pipeline per core:
  - Host precomputes y = x*p (elementwise prep, 0.01% of FLOPs) and splits
    it into fp8e4m3 hi + lo residual streams, packed with a row pitch of
    136 so the DoubleRow Ko stride (2 rows = 272 elems) is 16-aligned.
    Partitions 0-63 hold rows 0..67, partitions 64-127 hold rows 1..68.
  - Weights are scaled x16 and split hi/lo in fp8; the ACT PSUM->SBUF
    copy unscales via its activation `scale` and adds bias.
  - One DoubleRow matmul contracts 4 taps at once: partition halves give
    row shifts {0,1}, the Ko pair dim (+272 elems = +2 rows) gives {2,3}
    (tap k=3 has zero weight). 3 DR matmuls cover the 9 taps per pass;
    3 passes (wh*yh, wh*yl, wl*yh) give ~bf16 accuracy at 0.5 cyc/row:
    9 DR matmuls x 128-free per output row = 576 PE cycles vs 768 bf16.
  - Small first/last row groups, grouped stores fanned across queues,
    two warm-up matmuls to open the PE clock-ramp window early.
"""

import sys

import numpy as np

sys.path.insert(0, "/opt/trn_rl_repo")

import ml_dtypes

import concourse.bass as bass
import concourse.mybir as mybir
from concourse.bass_utils import run_bass_kernel_spmd
from concourse.tile import TileContext

F32 = mybir.dt.float32
BF16 = mybir.dt.bfloat16
FP8 = mybir.dt.float8e4

B, C, O, H, W = 4, 64, 64, 128, 128
HS = 64  # output rows per core
WP2 = 136  # padded row pitch (130 data cols, padded so 2 rows % 16 == 0)
NR = 68  # rows per stored half-slab
FREE3 = NR * WP2  # 9248
KO = 2 * WP2  # 272: DoubleRow Ko stride (+2 rows)
WSCALE = 16.0
N_WARM = 2
# y-stream chunk edges in rows (both hi and lo streams)
ROW_EDGES = [0, 6, 14, 22, 32, 42, 52, 62, 68]
# matmul groups: (start_row, n_rows)
MM_CHUNKS = (
    [(0, 4), (4, 4)]
    + [(8 + 4 * i, 4) for i in range(13)]
    + [(62, 2), (60, 2)]
)
STORE_GROUPS = [
    (0, 3072),
    (3072, 5120),
    (5120, 6656),
    (6656, 7168),
    (7168, 7680),
    (7680, 7936),
]

_cache: dict = {}


def _program() -> bass.Bass:
    from concourse.bacc import Bacc

    nc = Bacc()
    yh_h = nc.dram_tensor("yh", [128, FREE3], FP8, kind="ExternalInput")
    yl_h = nc.dram_tensor("yl", [128, FREE3], FP8, kind="ExternalInput")
    # w8: blocks (s, l): cols (s*3+l)*128 + two*64 + o; s=0 hi, s=1 lo
    w_h = nc.dram_tensor("w", [128, 768], FP8, kind="ExternalInput")
    bias_h = nc.dram_tensor("bias", [O, 1], F32, kind="ExternalInput")
    out_h = nc.dram_tensor("out", [O, HS * W], F32, kind="ExternalOutput")

    with TileContext(nc) as tc:
        with (
            tc.tile_pool(name="const", bufs=1) as cpool,
            tc.tile_pool(name="work", bufs=1) as wpool,
            tc.tile_pool(name="psum", bufs=4, space="PSUM") as ppool,
        ):
            # PE warm-up to start the clock-ramp window early
            wrm_w = cpool.tile([1, 1], BF16)
            wrm_r = cpool.tile([1, 512], BF16)
            nc.vector.memset(wrm_w[:, :], 0.0)
            nc.vector.memset(wrm_r[:, :], 0.0)
            for _ in range(N_WARM):
                pw = ppool.tile([O, 512], F32)
                nc.tensor.matmul(
                    pw[0:1, :], wrm_w[:, :], wrm_r[:, :], start=True, stop=True
                )

            wt = cpool.tile([128, 768], FP8)
            bt = cpool.tile([O, 1], F32)
            yh = wpool.tile([128, FREE3], FP8)
            yl = wpool.tile([128, FREE3], FP8)
            st = wpool.tile([O, HS * W], F32)

            # weights/bias via Pool SWDGE (no HWDGE slot)
            nc.gpsimd.dma_start(out=wt[:, :], in_=w_h[:, :])
            nc.gpsimd.dma_start(out=bt[:, :], in_=bias_h[:, :])

            for j in range(len(ROW_EDGES) - 1):
                c0 = ROW_EDGES[j] * WP2
                c1 = ROW_EDGES[j + 1] * WP2
                nc.sync.dma_start(out=yh[:, c0:c1], in_=yh_h[:, c0:c1])
                nc.sync.dma_start(out=yl[:, c0:c1], in_=yl_h[:, c0:c1])

            # lhsT blocks: [p, block (s*3+l), two, o]
            w4 = wt[:].rearrange("p (b two o) -> p b two o", b=6, two=2)
            yh3 = yh[:].rearrange("p (r c) -> p r c", r=NR)
            yl3 = yl[:].rearrange("p (r c) -> p r c", r=NR)

            def rhs(stream3, m, l):
                # rows {m, m+2} via step-2 slice = the DoubleRow Ko pair
                return stream3[0:128, m : m + 3 : 2, l : l + W]

            # yl-dependent hl pass last within each row: the PE exec queue
            # is FIFO, so a late yl chunk must not block yh-only matmuls
            # pass-major emission: all yh-dependent matmuls (hh, lh) of the
            # whole group run before the yl-dependent hl pass, so a late yl
            # chunk never blocks yh-only work in the FIFO PE queue. PSUM
            # start=True zeroes the whole 2KB bank (lazily, per first write),
            # so exactly ONE start for the tile; later slices accumulate
            # from the pending-zero state with start=False.
            passes = ((0, yh3), (1, yh3), (0, yl3))
            for R, r in MM_CHUNKS:
                ps = ppool.tile([O, r * W], F32)
                first = True
                for pi, (s, ystream) in enumerate(passes):
                    for i in range(r):
                        m = R + i
                        sl = ps[:, W * i : W * (i + 1)]
                        for l in range(3):
                            nc.tensor.matmul(
                                sl,
                                w4[0:128, 3 * s + l, 0:2, 0:64],
                                rhs(ystream, m, l),
                                start=first,
                                stop=(pi == 2 and i == r - 1 and l == 2),
                                perf_mode=mybir.MatmulPerfMode.DoubleRow,
                                skip_group_check=True,
                            )
                            first = False
                # unscale (1/16) + bias while copying PSUM -> SBUF staging
                ss = st[:, W * R : W * (R + r)]
                if R == 62:
                    nc.vector.tensor_scalar(
                        out=ss, in0=ps[:, :], scalar1=1.0 / WSCALE,
                        scalar2=bt[:, 0:1], op0=mybir.AluOpType.mult,
                        op1=mybir.AluOpType.add,
                    )
                else:
                    nc.scalar.activation(
                        ss, ps[:, :], mybir.ActivationFunctionType.Identity,
                        bias=bt[:, 0:1], scale=1.0 / WSCALE,
                    )
            for g0, g1 in STORE_GROUPS:
                nc.sync.dma_start(out=out_h[:, g0:g1], in_=st[:, g0:g1])
            # tail stores on different queues so their issue overlaps
            nc.gpsimd.dma_start(out=out_h[:, 7936:8064], in_=st[:, 7936:8064])
            nc.scalar.dma_start(out=out_h[:, 8064:8192], in_=st[:, 8064:8192])
    return nc


def _pack_weights(wt):
    """[O,C,3,3] -> [128, 768] fp8: blocks (s,l), s=0: fp8(16w) hi,
    s=1: fp8 residual; partition p = channel p%64 with row-shift p//64;
    two-slot j selects tap k = p//64 + 2j (k=3 -> 0)."""
    w16 = wt.astype(np.float64).transpose(1, 2, 3, 0) * WSCALE  # [c,k,l,o]
    wk = np.zeros((C, 4, 3, O), np.float64)
    wk[:, 0:3] = w16
    hi = np.asarray(wk.astype(ml_dtypes.float8_e4m3), np.float64)
    lo = (wk - hi).astype(ml_dtypes.float8_e4m3)
    out = np.zeros((128, 768), ml_dtypes.float8_e4m3)
    for s, ww in ((0, hi.astype(ml_dtypes.float8_e4m3)), (1, lo)):
        for l in range(3):
            for h in range(2):
                for j in range(2):
                    out[64 * h : 64 * h + 64,
                        (3 * s + l) * 128 + 64 * j : (3 * s + l) * 128 + 64 * j + 64,
                        ] = np.asarray(ww)[:, h + 2 * j, l, :]
    return np.ascontiguousarray(out)


def kernel(inputs, alpha, weight, bias, a, b, c):
    x = np.asarray(inputs, np.float32)
    al = np.asarray(alpha, np.float32)
    wt = np.asarray(weight, np.float32)
    bs = np.asarray(bias, np.float32)
    av, bv, cv = float(a), float(b), float(c)

    if "nc" not in _cache:
        nc_new = _program()
        nc_new.finalize()
        _cache["nc"] = nc_new
    nc = _cache["nc"]

    w_packed = _pack_weights(wt)
    b_packed = np.ascontiguousarray(bs.reshape(O, 1))

    in_maps = []
    for core in range(8):
        b_idx, hh = divmod(core, 2)
        r0 = hh * HS - 1  # global row of padded row 0
        ys = np.zeros((C, NR + 1, WP2), np.float32)
        als = np.zeros((1, NR + 1, WP2), np.float32)
        lo_r = max(0, r0)
        hi_r = min(H, r0 + HS + 2)
        ys[:, lo_r - r0 : hi_r - r0, 1 : 1 + W] = x[b_idx, :, lo_r:hi_r, :]
        als[:, lo_r - r0 : hi_r - r0, 1 : 1 + W] = al[b_idx, :, lo_r:hi_r, :]
        y = ys * ((av * als + bv) * als + cv)  # y = x * p
        y_hi = y.astype(ml_dtypes.float8_e4m3)
        y_lo = (y - np.asarray(y_hi, np.float32)).astype(ml_dtypes.float8_e4m3)

        def pack(yv):
            return np.concatenate(
                [yv[:, 0:NR].reshape(C, FREE3), yv[:, 1 : NR + 1].reshape(C, FREE3)],
                axis=0,
            )

        in_maps.append(
            {
                "yh": np.ascontiguousarray(pack(y_hi)),
                "yl": np.ascontiguousarray(pack(y_lo)),
                "w": w_packed,
                "bias": b_packed,
            }
        )

    res = run_bass_kernel_spmd(nc, in_maps, list(range(8)))

    out = np.empty((B, O, H, W), np.float32)
    for core in range(8):
        b_idx, hh = divmod(core, 2)
        out[b_idx, :, hh * HS : (hh + 1) * HS, :] = res.results[core]["out"].reshape(
            O, HS, W
        )
    return out
